# revision 10
# baseline (speedup 1.0000x reference)
"""Self-contained Trainium2 Bass kernel for nn_ANEDecoderLayer (ANE decoder layer).

Shapes (hardcoded): B=2, C=2048, S=1024, H=16, HD=128, FF=8192, fp32 I/O.

Sharding: hybrid batch(2) x tensor-parallel(4) over 8 NeuronCores.
  core = g*4 + r:  g = batch index, r = TP rank.
  Within each group of 4 cores: heads sharded 4/core, d_ff sharded 2048/core.
  Block outputs (row-split Wo / W_down partial sums) are AllReduced in bf16
  within each 4-core group: replica_groups [[0,1,2,3],[4,5,6,7]].

Host-side preprocessing:
  - RMSNorm weights folded into the following matmul weights.
  - Weights pre-transposed, pre-tiled for lhsT layout, cast to bf16.
  - sin_k/cos_k pre-scaled by 1/sqrt(HD) (folds attention scale into K).
  - kv cache scatter (kv_write_idx) folded into a row-permutation of the mask.
  - additive masks converted to multiplicative exp(mask) tile patterns with
    per-tile classification (all-ones -> no op, all-zero -> tile skipped,
    else multiply by a deduplicated pattern tile).

Compute: matmuls in bf16 (fp32 PSUM accumulation); RMSNorm statistics via an
all-ones stationary matmul in float32r (sum + partition-broadcast fused, 1
cycle/row); softmax denominators likewise summed+broadcast with an all-ones
bf16 stationary; V is produced directly in [k-position, head*HD] layout by
making the normalized-activation chunk the stationary operand (no PE
transposes); softmax without max-subtraction (scores are O(5); exp(-1e9)=0
handled by tile skipping); residual stream held in bf16 in SBUF.
"""

import numpy as np
import ml_dtypes

import concourse.mybir as mybir
import concourse.tile as tile
from concourse import bacc
from concourse.bass_utils import run_bass_kernel_spmd

# ---------------------------------------------------------------- constants
B, C, S, H, HD, FF = 2, 2048, 1024, 16, 128, 8192
EPS = 1e-5
SCALE = 1.0 / float(np.sqrt(HD))

NCORES = 8
TPG = 4                      # tensor-parallel group size
HPC = H // TPG               # heads per core = 4
OCA = HPC * HD               # attention out-channels per core = 512
FFC = FF // TPG              # ff channels per core = 2048

CT = C // 128                # 16 c-chunks
ST = S // 512                # 2 s-chunks of 512
KT = S // 128                # 8 k-chunks
FFT = FFC // 128             # 16 ff-chunks per core

F32 = mybir.dt.float32
F32R = mybir.dt.float32r
BF = mybir.dt.bfloat16
AF = mybir.ActivationFunctionType
MULT = mybir.AluOpType.mult
ADD = mybir.AluOpType.add
BF_NP = ml_dtypes.bfloat16

REPLICA_GROUPS = [[0, 1, 2, 3], [4, 5, 6, 7]]

_CACHE: dict = {}


# ---------------------------------------------------------------- host prep
def _pack_lhsT(wT: np.ndarray) -> np.ndarray:
    """wT: (K, M) contraction-major weight. Returns (M//128, 128, K) bf16 where
    pack[m][p, kc*128+f] = wT[kc*128+p, m*128+f]; a DMA of pack[m] gives an
    SBUF tile whose slice [:, kc*128:(kc+1)*128] is the lhsT for contraction
    chunk kc -> output chunk m."""
    K, M = wT.shape
    Kt, Mt = K // 128, M // 128
    t = wT.reshape(Kt, 128, Mt, 128)              # [kc, p, m, f]
    t = t.transpose(2, 1, 0, 3).reshape(Mt, 128, K)
    return np.ascontiguousarray(t.astype(BF_NP))


def _classify_mask(mask_eff: np.ndarray):
    """mask_eff: (S, S) additive mask, (k, q) orientation. Returns
    (cls, patterns): cls[qc][kc] in {'c' (clean), 's' (skip), int idx};
    patterns: (NB, 128, 512) bf16 multiplicative tiles."""
    mm = np.exp(np.minimum(mask_eff.astype(np.float64), 0.0)).astype(np.float32)
    # positive masks would overflow exp; reference masks are <= 0
    if mask_eff.max() > 0:
        mm = np.exp(mask_eff.astype(np.float64)).astype(np.float32)
    patterns = []
    keys = {}
    cls = [[None] * KT for _ in range(ST)]
    for qc in range(ST):
        for kc in range(KT):
            sub = mm[kc * 128:(kc + 1) * 128, qc * 512:(qc + 1) * 512]
            if np.all(sub == 1.0):
                cls[qc][kc] = 'c'
            elif np.all(sub == 0.0):
                cls[qc][kc] = 's'
            else:
                kb = sub.tobytes()
                if kb not in keys:
                    keys[kb] = len(patterns)
                    patterns.append(sub.astype(BF_NP))
                cls[qc][kc] = keys[kb]
    if patterns:
        pat = np.stack(patterns)
    else:
        pat = np.zeros((1, 128, 512), BF_NP)
    return cls, pat


def _prep_host(inputs):
    """Returns (shared_map, per_rank_maps, sa_cls, ca_cls)."""
    g = lambda k: np.asarray(inputs[k], dtype=np.float32)

    sinq = np.ascontiguousarray(g('sin_q').reshape(HD, S))
    cosq = np.ascontiguousarray(g('cos_q').reshape(HD, S))
    sink = np.ascontiguousarray(g('sin_k').reshape(HD, S) * SCALE)
    cosk = np.ascontiguousarray(g('cos_k').reshape(HD, S) * SCALE)

    idx = np.asarray(inputs['kv_write_idx']).astype(np.int64)
    if not np.array_equal(np.sort(idx), np.arange(S)):
        raise NotImplementedError("kv_write_idx must be a permutation of arange(S)")
    sa_mask = g('self_attn_mask').reshape(S, S)[idx, :]     # effective (k, q) mask
    ca_mask = g('cross_attn_mask').reshape(S, S)
    sa_cls, sa_pat = _classify_mask(sa_mask)
    ca_cls, ca_pat = _classify_mask(ca_mask)

    P_rot = np.zeros((HD, HD), np.float32)
    P_rot[np.arange(64), np.arange(64, 128)] = -1.0
    P_rot[np.arange(64, 128), np.arange(64)] = 1.0

    shared = {
        'sinq': sinq.astype(BF_NP), 'cosq': cosq.astype(BF_NP),
        'sink': sink.astype(BF_NP), 'cosk': cosk.astype(BF_NP),
        'ones_mat_bf': np.ones((128, 128), BF_NP),
        'protT': np.ascontiguousarray(P_rot.T).astype(BF_NP),
        'mask_sa': sa_pat, 'mask_ca': ca_pat,
    }

    w_sa, w_ca, w_mlp = g('w_sa'), g('w_ca'), g('w_mlp')
    per_rank = []
    for r in range(TPG):
        asl = slice(r * OCA, (r + 1) * OCA)
        fsl = slice(r * FFC, (r + 1) * FFC)
        m = {}
        for tag in ('sa', 'ca'):
            wnorm = w_sa if tag == 'sa' else w_ca
            for p in ('q', 'k'):
                W = g(f'w{p}_{tag}')[asl, :] * wnorm[None, :]
                m[f'w{p}_{tag}'] = _pack_lhsT(np.ascontiguousarray(W.T))
            Wv = g(f'wv_{tag}')[asl, :] * wnorm[None, :]
            # moving-operand layout: (CT, 128, OCA); partition = c chunk
            m[f'wvT_{tag}'] = np.ascontiguousarray(
                Wv.T.reshape(CT, 128, OCA).astype(BF_NP))
            Wo = g(f'wo_{tag}')[:, asl]
            m[f'wo_{tag}'] = _pack_lhsT(np.ascontiguousarray(Wo.T))
        for p, key in (('g', 'w_gate'), ('u', 'w_up')):
            W = g(key)[fsl, :] * w_mlp[None, :]
            m[f'w{p}'] = _pack_lhsT(np.ascontiguousarray(W.T))
        Wd = g('w_down')[:, fsl]
        m['wd'] = _pack_lhsT(np.ascontiguousarray(Wd.T))
        per_rank.append(m)

    return shared, per_rank, sa_cls, ca_cls


# ---------------------------------------------------------------- builder
def _build(sa_cls, ca_cls, nb_sa, nb_ca):
    nc = bacc.Bacc("TRN2", target_bir_lowering=False, debug=False,
                   num_devices=NCORES)

    d_x = nc.declare_dram_parameter("x", [C, S], BF, isOutput=False)
    d_tab = {k: nc.declare_dram_parameter(k, [HD, S], BF, isOutput=False)
             for k in ('sinq', 'cosq', 'sink', 'cosk')}
    d_omb = nc.declare_dram_parameter("ones_mat_bf", [128, 128], BF, isOutput=False)
    d_pr = nc.declare_dram_parameter("protT", [128, 128], BF, isOutput=False)
    d_msa = nc.declare_dram_parameter("mask_sa", [nb_sa, 128, 512], BF, isOutput=False)
    d_mca = nc.declare_dram_parameter("mask_ca", [nb_ca, 128, 512], BF, isOutput=False)
    d_w = {}
    for t in ('sa', 'ca'):
        for p in ('q', 'k'):
            d_w[f'w{p}_{t}'] = nc.declare_dram_parameter(
                f'w{p}_{t}', [OCA // 128, 128, C], BF, isOutput=False)
        d_w[f'wvT_{t}'] = nc.declare_dram_parameter(
            f'wvT_{t}', [CT, 128, OCA], BF, isOutput=False)
        d_w[f'wo_{t}'] = nc.declare_dram_parameter(
            f'wo_{t}', [CT, 128, OCA], BF, isOutput=False)
    for k in ('wg', 'wu', 'wd'):
        kdim = C if k != 'wd' else FFC
        d_w[k] = nc.declare_dram_parameter(k, [FFT, 128, kdim], BF, isOutput=False)
    d_out = nc.declare_dram_parameter("out", [C, S], F32, isOutput=True)

    with tile.TileContext(nc) as tc:
        with (
            tc.tile_pool(name="const", bufs=1) as cpool,
            tc.tile_pool(name="xp", bufs=1) as xpool,
            tc.tile_pool(name="hp", bufs=1) as hpool,
            tc.tile_pool(name="wb", bufs=6) as wpool,
            tc.tile_pool(name="oo", bufs=3) as opool,
            tc.tile_pool(name="sm", bufs=2) as spool,
            tc.tile_pool(name="dram", bufs=1, space="DRAM") as dpool,
            tc.tile_pool(name="psA", bufs=7, space="PSUM") as psA,
        ):
            # ---------------- constants / tables ----------------
            def ptile(pool, shape, dt, name):
                return pool.tile(shape, dt, name=name, tag=name)

            xt = [ptile(xpool, [128, S], BF, f"x{cc}") for cc in range(CT)]
            for cc in range(CT):
                nc.sync.dma_start(xt[cc][:], d_x.ap()[cc * 128:(cc + 1) * 128, :])

            ones_mat_bf = ptile(cpool, [128, 128], BF, "ones_mat_bf")
            protT = ptile(cpool, [128, 128], BF, "protT")
            eps_t = ptile(cpool, [128, 1], F32, "eps_t")
            nc.sync.dma_start(ones_mat_bf[:], d_omb.ap())
            nc.sync.dma_start(protT[:], d_pr.ap())
            nc.vector.memset(eps_t[:], EPS)
            tabs = {}
            for k in d_tab:
                tabs[k] = ptile(cpool, [HD, S], BF, f"tab_{k}")
                nc.sync.dma_start(tabs[k][:], d_tab[k].ap())
            used_sa = {c for row in sa_cls for c in row if isinstance(c, int)}
            used_ca = {c for row in ca_cls for c in row if isinstance(c, int)}
            msk_sa, msk_ca = {}, {}
            for i in sorted(used_sa):
                msk_sa[i] = ptile(cpool, [128, 512], BF, f"msa{i}")
                nc.sync.dma_start(msk_sa[i][:], d_msa.ap()[i])
            for i in sorted(used_ca):
                msk_ca[i] = ptile(cpool, [128, 512], BF, f"mca{i}")
                nc.sync.dma_start(msk_ca[i][:], d_mca.ap()[i])

            # ---------------- residual stream x ----------------
            ht = [ptile(hpool, [128, S], BF, f"h{cc}") for cc in range(CT)]

            # ---------------- helpers ----------------
            def norm_sc(sc, scope):
                """ht[:, s0] = xt[:, s0] * rsqrt(mean_c(xt^2) + eps).
                Sum over C and partition-broadcast fused into one f32r
                matmul chain with an all-ones stationary."""
                s0 = slice(sc * 512, (sc + 1) * 512)
                with nc.named_scope(scope):
                    ss = psA.tile([128, 512], F32, tag="acc")
                    for cc in range(CT):
                        sq = spool.tile([128, 512], BF, tag="sq")
                        nc.scalar.activation(sq[:], xt[cc][:, s0], AF.Square)
                        nc.tensor.matmul(ss[:], ones_mat_bf[:], sq[:],
                                         start=(cc == 0), stop=(cc == CT - 1))
                    rs = spool.tile([128, 512], F32, tag="rs")
                    nc.scalar.activation(rs[:], ss[:], AF.Sqrt,
                                         bias=eps_t[:], scale=1.0 / C)
                    rr = spool.tile([128, 512], F32, tag="rr")
                    nc.vector.reciprocal_approx_fast(rr[:], rs[:])
                    for cc in range(CT):
                        nc.vector.tensor_tensor(ht[cc][:, s0], xt[cc][:, s0],
                                                rr[:], op=MULT)

            def res_sc(b_half, sc, scope, final=False):
                """xt[:, s0] += AR half (bf16 dram (C,512)); final -> write out."""
                s0 = slice(sc * 512, (sc + 1) * 512)
                with nc.named_scope(scope):
                    for cc in range(CT):
                        ar = opool.tile([128, 512], BF, tag="ar")
                        nc.sync.dma_start(ar[:], b_half[cc * 128:(cc + 1) * 128, :])
                        if final:
                            ot = opool.tile([128, 512], F32, tag="obuf")
                            nc.vector.tensor_tensor(ot[:], xt[cc][:, s0], ar[:],
                                                    op=ADD)
                            nc.sync.dma_start(
                                d_out.ap()[cc * 128:(cc + 1) * 128, s0], ot[:])
                        else:
                            nc.vector.tensor_tensor(xt[cc][:, s0], xt[cc][:, s0],
                                                    ar[:], op=ADD)

            def attention(t, cls, msk, apool, b_prev):
                """One attention block. b_prev: previous block's AR halves (or
                None); its residual is applied lazily per s-chunk here so the
                previous AllReduce overlaps this block's per-chunk compute.
                Returns this block's AR output halves."""
                qk_rope, vTc = {}, {}
                att = [apool.tile([128, S], BF, name=f"att{t}{oc}",
                                  tag=f"att{oc}", bufs=1) for oc in range(HPC)]
                b_in = [dpool.tile([C, 512], BF, name=f"bin_{t}{h}",
                                   tag=f"bin_{t}{h}") for h in range(ST)]
                b_out = [dpool.tile([C, 512], BF, name=f"bout_{t}{h}",
                                    tag=f"bout_{t}{h}") for h in range(ST)]
                # per-block weight preloads (wo + wvT, reused across halves);
                # DMAs emitted lazily at first use point to keep the queue
                # order aligned with consumption order.
                wvt, wot = {}, {}

                def load_wvt():
                    for cc in range(CT):
                        wvt[cc] = apool.tile([128, OCA], BF, name=f"wvT{t}{cc}",
                                             tag=f"wvT{cc}", bufs=1)
                        nc.sync.dma_start(wvt[cc][:], d_w[f'wvT_{t}'].ap()[cc])

                def load_wot():
                    for cc in range(CT):
                        wot[cc] = apool.tile([128, OCA], BF, name=f"wo{t}{cc}",
                                             tag=f"wo{cc}", bufs=1)
                        nc.sync.dma_start(wot[cc][:], d_w[f'wo_{t}'].ap()[cc])
                # per q-half: attention core, then immediately wo + AllReduce
                # for that s-half so the collective overlaps the other half's
                # attention (engine instruction streams are static - emission
                # order is execution order per engine).
                def attn_wo_qc(qc):
                    if not wot:
                        load_wot()
                    s0 = slice(qc * 512, (qc + 1) * 512)
                    with nc.named_scope(f"{t}_attn"):
                        for oc in range(HPC):
                            qr, kr = qk_rope[('q', oc)], qk_rope[('k', oc)]
                            valid = [kc for kc in range(KT) if cls[qc][kc] != 's']
                            probs = {}
                            for kc in valid:
                                sp = psA.tile([128, 512], F32, tag="acc")
                                nc.tensor.matmul(
                                    sp[:], kr[:, kc * 128:(kc + 1) * 128],
                                    qr[:, s0], start=True, stop=True)
                                pt = apool.tile([128, 512], BF, tag="probs",
                                                bufs=10)
                                nc.scalar.activation(pt[:], sp[:], AF.Exp)
                                if cls[qc][kc] != 'c':
                                    nc.vector.tensor_tensor(
                                        pt[:], pt[:], msk[cls[qc][kc]][:], op=MULT)
                                probs[kc] = pt
                            # denominator summed over k AND broadcast to all
                            # 128 partitions via the all-ones stationary
                            dnb = psA.tile([128, 512], F32, tag="acc")
                            for i, kc in enumerate(valid):
                                nc.tensor.matmul(dnb[:], ones_mat_bf[:],
                                                 probs[kc][:],
                                                 start=(i == 0),
                                                 stop=(i == len(valid) - 1))
                            rbs = spool.tile([128, 512], F32, tag="rbs")
                            nc.vector.reciprocal_approx_fast(rbs[:], dnb[:])
                            pa = psA.tile([128, 512], F32, tag="acc")
                            for i, kc in enumerate(valid):
                                nc.tensor.matmul(
                                    pa[:], vTc[kc][:, oc * 128:(oc + 1) * 128],
                                    probs[kc][:],
                                    start=(i == 0), stop=(i == len(valid) - 1))
                            nc.vector.tensor_tensor(att[oc][:, s0], pa[:], rbs[:],
                                                    op=MULT)
                    with nc.named_scope(f"{t}_wo"):
                        for cc in range(CT):
                            ps = psA.tile([128, 512], F32, tag="acc")
                            for ac in range(HPC):
                                nc.tensor.matmul(
                                    ps[:], wot[cc][:, ac * 128:(ac + 1) * 128],
                                    att[ac][:, s0],
                                    start=(ac == 0), stop=(ac == HPC - 1))
                            osb = opool.tile([128, 512], BF, tag="obuf")
                            nc.scalar.activation(osb[:], ps[:], AF.Copy)
                            nc.sync.dma_start(
                                b_in[qc][cc * 128:(cc + 1) * 128, :], osb[:])
                        nc.gpsimd.collective_compute(
                            "AllReduce", ADD, replica_groups=REPLICA_GROUPS,
                            ins=[b_in[qc][:].opt()], outs=[b_out[qc][:].opt()])

                # causal early path: if every non-skip key tile for q-half 0
                # lies in s-half 0, its attention + wo + AllReduce can be
                # emitted before s-half 1's projections exist.
                early = all(kc < KT // 2 for kc in range(KT)
                            if cls[0][kc] != 's')
                for sc in range(ST):
                    s0 = slice(sc * 512, (sc + 1) * 512)
                    if b_prev is not None:
                        # scheduler-only fence: keep every engine's queue
                        # order aligned with emission order here, so ops
                        # depending on the previous block's AllReduce can't
                        # be hoisted ahead of this block's independent work
                        # (head-of-line blocking on the strict-FIFO queues).
                        tc.no_sync_barrier()
                        res_sc(b_prev[sc], sc, f"{t}_res")
                    norm_sc(sc, f"{t}_norm")
                    with nc.named_scope(f"{t}_qkv"):
                        for oc in range(HPC):
                            if ('k', oc) not in qk_rope:
                                qk_rope[('k', oc)] = apool.tile(
                                    [128, S], BF, name=f"kr{t}{oc}",
                                    tag=f"kr{oc}", bufs=1)
                            wsb = wpool.tile([128, C], BF, tag="wbig")
                            nc.sync.dma_start(wsb[:], d_w[f'wk_{t}'].ap()[oc])
                            ps = psA.tile([128, 512], F32, tag="acc")
                            for cc in range(CT):
                                nc.tensor.matmul(
                                    ps[:], wsb[:, cc * 128:(cc + 1) * 128],
                                    ht[cc][:, s0],
                                    start=(cc == 0), stop=(cc == CT - 1))
                            lin = spool.tile([128, 512], BF, tag="lin")
                            nc.scalar.activation(lin[:], ps[:], AF.Copy)
                            rot = psA.tile([128, 512], F32, tag="acc")
                            nc.tensor.matmul(rot[:], protT[:], lin[:],
                                             start=True, stop=True)
                            dst = qk_rope[('k', oc)]
                            nc.vector.tensor_tensor(
                                dst[:, s0], lin[:], tabs['cosk'][:, s0], op=MULT)
                            s2 = spool.tile([128, 512], BF, tag="rsc")
                            nc.vector.tensor_tensor(
                                s2[:], rot[:], tabs['sink'][:, s0], op=MULT)
                            nc.vector.tensor_tensor(
                                dst[:, s0], dst[:, s0], s2[:], op=ADD)
                        # V directly in [k-pos, oc*HD] layout: stationary =
                        # normalized-activation chunk, moving = WvT chunk
                        if not wvt:
                            load_wvt()
                        for j in range(4):
                            kc = sc * 4 + j
                            vTc[kc] = apool.tile([128, OCA], BF,
                                                 name=f"vT{t}{kc}",
                                                 tag=f"vT{kc}", bufs=1)
                            ps = psA.tile([128, 512], F32, tag="acc")
                            sblk = slice(sc * 512 + j * 128,
                                         sc * 512 + (j + 1) * 128)
                            for cc in range(CT):
                                nc.tensor.matmul(ps[:], ht[cc][:, sblk],
                                                 wvt[cc][:],
                                                 start=(cc == 0),
                                                 stop=(cc == CT - 1))
                            nc.scalar.activation(vTc[kc][:], ps[:], AF.Copy)
                        for oc in range(HPC):
                            if ('q', oc) not in qk_rope:
                                qk_rope[('q', oc)] = apool.tile(
                                    [128, S], BF, name=f"qr{t}{oc}",
                                    tag=f"qr{oc}", bufs=1)
                            wsb = wpool.tile([128, C], BF, tag="wbig")
                            nc.sync.dma_start(wsb[:], d_w[f'wq_{t}'].ap()[oc])
                            ps = psA.tile([128, 512], F32, tag="acc")
                            for cc in range(CT):
                                nc.tensor.matmul(
                                    ps[:], wsb[:, cc * 128:(cc + 1) * 128],
                                    ht[cc][:, s0],
                                    start=(cc == 0), stop=(cc == CT - 1))
                            lin = spool.tile([128, 512], BF, tag="lin")
                            nc.scalar.activation(lin[:], ps[:], AF.Copy)
                            rot = psA.tile([128, 512], F32, tag="acc")
                            nc.tensor.matmul(rot[:], protT[:], lin[:],
                                             start=True, stop=True)
                            dst = qk_rope[('q', oc)]
                            nc.vector.tensor_tensor(
                                dst[:, s0], lin[:], tabs['cosq'][:, s0], op=MULT)
                            s2 = spool.tile([128, 512], BF, tag="rsc")
                            nc.vector.tensor_tensor(
                                s2[:], rot[:], tabs['sinq'][:, s0], op=MULT)
                            nc.vector.tensor_tensor(
                                dst[:, s0], dst[:, s0], s2[:], op=ADD)
                    if sc == 0 and early:
                        attn_wo_qc(0)
                for qc in range(ST):
                    if qc == 0 and early:
                        continue
                    attn_wo_qc(qc)
                return b_out

            # ================= attention blocks =================
            with tc.tile_pool(name="ap", bufs=1) as apool:
                b_sa = attention('sa', sa_cls, msk_sa, apool, None)
                b_ca = attention('ca', ca_cls, msk_ca, apool, b_sa)

            # ================= MLP =================
            mpool_ctx = tc.tile_pool(name="mp", bufs=1)
            mpool = mpool_ctx.__enter__()
            gact = [mpool.tile([128, S], BF, name=f"gact{f}", tag=f"gact{f}",
                               bufs=1) for f in range(FFT)]
            # down/AR in chunks [512, 256, 256]: the small tail chunks keep
            # the last exposed AllReduce (and its residual) short.
            CH = [(0, 512), (512, 256), (768, 256)]
            b_in = [dpool.tile([C, w], BF, name=f"bin_mlp{i}",
                               tag=f"bin_mlp{i}") for i, (lo, w) in enumerate(CH)]
            b_out = [dpool.tile([C, w], BF, name=f"bout_mlp{i}",
                                tag=f"bout_mlp{i}") for i, (lo, w) in enumerate(CH)]

            def mlp_up(sc):
                s0 = slice(sc * 512, (sc + 1) * 512)
                with nc.named_scope("mlp_up"):
                    for f in range(FFT):
                        wg = wpool.tile([128, C], BF, tag="wbig")
                        nc.sync.dma_start(wg[:], d_w['wg'].ap()[f])
                        wu = wpool.tile([128, C], BF, tag="wbig")
                        nc.sync.dma_start(wu[:], d_w['wu'].ap()[f])
                        pg = psA.tile([128, 512], F32, tag="acc")
                        for cc in range(CT):
                            nc.tensor.matmul(pg[:], wg[:, cc * 128:(cc + 1) * 128],
                                             ht[cc][:, s0],
                                             start=(cc == 0), stop=(cc == CT - 1))
                        pu = psA.tile([128, 512], F32, tag="acc")
                        for cc in range(CT):
                            nc.tensor.matmul(pu[:], wu[:, cc * 128:(cc + 1) * 128],
                                             ht[cc][:, s0],
                                             start=(cc == 0), stop=(cc == CT - 1))
                        gs = spool.tile([128, 512], BF, tag="lin")
                        nc.scalar.activation(gs[:], pg[:], AF.Silu)
                        nc.vector.tensor_tensor(gact[f][:, s0], gs[:], pu[:],
                                                op=MULT)

            def mlp_down_chunk(i):
                lo, w = CH[i]
                sl = slice(lo, lo + w)
                with nc.named_scope("mlp_down"):
                    for cc in range(CT):
                        wd = wpool.tile([128, FFC], BF, tag="wbig")
                        nc.sync.dma_start(wd[:], d_w['wd'].ap()[cc])
                        ps = psA.tile([128, w], F32, tag="acc")
                        for f in range(FFT):
                            nc.tensor.matmul(ps[:], wd[:, f * 128:(f + 1) * 128],
                                             gact[f][:, sl],
                                             start=(f == 0), stop=(f == FFT - 1))
                        osb = opool.tile([128, w], BF, tag="obuf")
                        nc.scalar.activation(osb[:], ps[:], AF.Copy)
                        nc.sync.dma_start(
                            b_in[i][cc * 128:(cc + 1) * 128, :], osb[:])
                nc.gpsimd.collective_compute(
                    "AllReduce", ADD, replica_groups=REPLICA_GROUPS,
                    ins=[b_in[i][:].opt()], outs=[b_out[i][:].opt()])

            def final_chunk(i, b):
                lo, w = CH[i]
                sl = slice(lo, lo + w)
                with nc.named_scope("mlp_res"):
                    for cc in range(CT):
                        ar = opool.tile([128, w], BF, tag="ar")
                        nc.sync.dma_start(ar[:], b[cc * 128:(cc + 1) * 128, :])
                        ot = opool.tile([128, w], F32, tag="fout")
                        nc.vector.tensor_tensor(ot[:], xt[cc][:, sl], ar[:],
                                                op=ADD)
                        nc.sync.dma_start(
                            d_out.ap()[cc * 128:(cc + 1) * 128, sl], ot[:])

            tc.no_sync_barrier()
            res_sc(b_ca[0], 0, "ca_res")
            norm_sc(0, "mlp_norm")
            mlp_up(0)
            mlp_down_chunk(0)
            tc.no_sync_barrier()
            res_sc(b_ca[1], 1, "ca_res")
            norm_sc(1, "mlp_norm")
            mlp_up(1)
            tc.no_sync_barrier()
            final_chunk(0, b_out[0])
            mlp_down_chunk(1)
            mlp_down_chunk(2)
            tc.no_sync_barrier()
            final_chunk(1, b_out[1])
            final_chunk(2, b_out[2])
            mpool_ctx.__exit__(None, None, None)

    nc.compile()
    return nc


# ---------------------------------------------------------------- entry
def _mask_sig(cls, pat):
    return (tuple(tuple(row) for row in cls), pat.tobytes())


def kernel(**inputs) -> np.ndarray:
    shared, per_rank, sa_cls, ca_cls = _prep_host(inputs)
    nb_sa, nb_ca = shared['mask_sa'].shape[0], shared['mask_ca'].shape[0]

    key = (_mask_sig(sa_cls, shared['mask_sa']),
           _mask_sig(ca_cls, shared['mask_ca']))
    if key not in _CACHE:
        _CACHE[key] = _build(sa_cls, ca_cls, nb_sa, nb_ca)
    nc = _CACHE[key]

    x = np.asarray(inputs['x'], dtype=np.float32)
    xb = [np.ascontiguousarray(x[g]).astype(BF_NP) for g in range(B)]
    in_maps = []
    for core in range(NCORES):
        g, r = core // TPG, core % TPG
        m = dict(shared)
        m['x'] = xb[g]
        m.update(per_rank[r])
        in_maps.append(m)

    res = run_bass_kernel_spmd(nc, in_maps, core_ids=list(range(NCORES)))
    out = np.stack([res.results[0]['out'], res.results[TPG]['out']], axis=0)
    return out.astype(np.float32)


# revision 15
# speedup vs baseline: 1.1899x; 1.1899x over previous
"""Self-contained Trainium2 Bass kernel for nn_ANEDecoderLayer (ANE decoder layer).

Shapes (hardcoded): B=2, C=2048, S=1024, H=16, HD=128, FF=8192, fp32 I/O.

Sharding: hybrid batch(2) x tensor-parallel(4) over 8 NeuronCores.
  core = g*4 + r:  g = batch index, r = TP rank.
  Within each group of 4 cores: heads sharded 4/core, d_ff sharded 2048/core.
  Block outputs (row-split Wo / W_down partial sums) are AllReduced in bf16
  within each 4-core group: replica_groups [[0,1,2,3],[4,5,6,7]].

Host-side preprocessing:
  - RMSNorm weights folded into the following matmul weights.
  - Weights pre-transposed, pre-tiled for lhsT layout, cast to bf16.
  - sin_k/cos_k pre-scaled by 1/sqrt(HD) (folds attention scale into K).
  - kv cache scatter (kv_write_idx) folded into a row-permutation of the mask.
  - additive masks converted to multiplicative exp(mask) tile patterns with
    per-tile classification (all-ones -> no op, all-zero -> tile skipped,
    else multiply by a deduplicated pattern tile).

Compute: matmuls in bf16 (fp32 PSUM accumulation); RMSNorm statistics via an
all-ones stationary matmul in float32r (sum + partition-broadcast fused, 1
cycle/row); softmax denominators likewise summed+broadcast with an all-ones
bf16 stationary; V is produced directly in [k-position, head*HD] layout by
making the normalized-activation chunk the stationary operand (no PE
transposes); softmax without max-subtraction (scores are O(5); exp(-1e9)=0
handled by tile skipping); residual stream held in bf16 in SBUF.
"""

import numpy as np
import ml_dtypes

import concourse.mybir as mybir
import concourse.tile as tile
from concourse import bacc
from concourse.bass_utils import run_bass_kernel_spmd

# ---------------------------------------------------------------- constants
B, C, S, H, HD, FF = 2, 2048, 1024, 16, 128, 8192
EPS = 1e-5
SCALE = 1.0 / float(np.sqrt(HD))

NCORES = 8
TPG = 4                      # tensor-parallel group size
HPC = H // TPG               # heads per core = 4
OCA = HPC * HD               # attention out-channels per core = 512
FFC = FF // TPG              # ff channels per core = 2048

CT = C // 128                # 16 c-chunks
ST = S // 512                # 2 s-chunks of 512
KT = S // 128                # 8 k-chunks
FFT = FFC // 128             # 16 ff-chunks per core

F32 = mybir.dt.float32
F32R = mybir.dt.float32r
BF = mybir.dt.bfloat16
AF = mybir.ActivationFunctionType
MULT = mybir.AluOpType.mult
ADD = mybir.AluOpType.add
BF_NP = ml_dtypes.bfloat16

REPLICA_GROUPS = [[0, 1, 2, 3], [4, 5, 6, 7]]

_CACHE: dict = {}


# ---------------------------------------------------------------- host prep
def _pack_lhsT(wT: np.ndarray) -> np.ndarray:
    """wT: (K, M) contraction-major weight. Returns (M//128, 128, K) bf16 where
    pack[m][p, kc*128+f] = wT[kc*128+p, m*128+f]; a DMA of pack[m] gives an
    SBUF tile whose slice [:, kc*128:(kc+1)*128] is the lhsT for contraction
    chunk kc -> output chunk m."""
    K, M = wT.shape
    Kt, Mt = K // 128, M // 128
    t = wT.reshape(Kt, 128, Mt, 128)              # [kc, p, m, f]
    t = t.transpose(2, 1, 0, 3).reshape(Mt, 128, K)
    return np.ascontiguousarray(t.astype(BF_NP))


def _classify_mask(mask_eff: np.ndarray):
    """mask_eff: (S, S) additive mask, (k, q) orientation. Returns
    (cls, patterns): cls[qc][kc] in {'c' (clean), 's' (skip), int idx};
    patterns: (NB, 128, 512) bf16 multiplicative tiles."""
    mm = np.exp(np.minimum(mask_eff.astype(np.float64), 0.0)).astype(np.float32)
    # positive masks would overflow exp; reference masks are <= 0
    if mask_eff.max() > 0:
        mm = np.exp(mask_eff.astype(np.float64)).astype(np.float32)
    patterns = []
    keys = {}
    cls = [[None] * KT for _ in range(ST)]
    for qc in range(ST):
        for kc in range(KT):
            sub = mm[kc * 128:(kc + 1) * 128, qc * 512:(qc + 1) * 512]
            if np.all(sub == 1.0):
                cls[qc][kc] = 'c'
            elif np.all(sub == 0.0):
                cls[qc][kc] = 's'
            else:
                kb = sub.tobytes()
                if kb not in keys:
                    keys[kb] = len(patterns)
                    patterns.append(sub.astype(BF_NP))
                cls[qc][kc] = keys[kb]
    if patterns:
        pat = np.stack(patterns)
    else:
        pat = np.zeros((1, 128, 512), BF_NP)
    return cls, pat


def _prep_host(inputs):
    """Returns (shared_map, per_rank_maps, sa_cls, ca_cls)."""
    g = lambda k: np.asarray(inputs[k], dtype=np.float32)

    sinq = np.ascontiguousarray(g('sin_q').reshape(HD, S))
    cosq = np.ascontiguousarray(g('cos_q').reshape(HD, S))
    sink = np.ascontiguousarray(g('sin_k').reshape(HD, S) * SCALE)
    cosk = np.ascontiguousarray(g('cos_k').reshape(HD, S) * SCALE)

    idx = np.asarray(inputs['kv_write_idx']).astype(np.int64)
    if not np.array_equal(np.sort(idx), np.arange(S)):
        raise NotImplementedError("kv_write_idx must be a permutation of arange(S)")
    sa_mask = g('self_attn_mask').reshape(S, S)[idx, :]     # effective (k, q) mask
    ca_mask = g('cross_attn_mask').reshape(S, S)
    sa_cls, sa_pat = _classify_mask(sa_mask)
    ca_cls, ca_pat = _classify_mask(ca_mask)

    P_rot = np.zeros((HD, HD), np.float32)
    P_rot[np.arange(64), np.arange(64, 128)] = -1.0
    P_rot[np.arange(64, 128), np.arange(64)] = 1.0

    shared = {
        'sinq': sinq.astype(BF_NP), 'cosq': cosq.astype(BF_NP),
        'sink': sink.astype(BF_NP), 'cosk': cosk.astype(BF_NP),
        'ones_mat_bf': np.ones((128, 128), BF_NP),
        'protT': np.ascontiguousarray(P_rot.T).astype(BF_NP),
        'mask_sa': sa_pat, 'mask_ca': ca_pat,
    }

    w_sa, w_ca, w_mlp = g('w_sa'), g('w_ca'), g('w_mlp')
    per_rank = []
    for r in range(TPG):
        asl = slice(r * OCA, (r + 1) * OCA)
        fsl = slice(r * FFC, (r + 1) * FFC)
        m = {}
        for tag in ('sa', 'ca'):
            wnorm = w_sa if tag == 'sa' else w_ca
            for p in ('q', 'k'):
                W = g(f'w{p}_{tag}')[asl, :] * wnorm[None, :]
                m[f'w{p}_{tag}'] = _pack_lhsT(np.ascontiguousarray(W.T))
            Wv = g(f'wv_{tag}')[asl, :] * wnorm[None, :]
            # moving-operand layout: (CT, 128, OCA); partition = c chunk
            m[f'wvT_{tag}'] = np.ascontiguousarray(
                Wv.T.reshape(CT, 128, OCA).astype(BF_NP))
            Wo = g(f'wo_{tag}')[:, asl]
            m[f'wo_{tag}'] = _pack_lhsT(np.ascontiguousarray(Wo.T))
        for p, key in (('g', 'w_gate'), ('u', 'w_up')):
            W = g(key)[fsl, :] * w_mlp[None, :]
            m[f'w{p}'] = _pack_lhsT(np.ascontiguousarray(W.T))
        Wd = g('w_down')[:, fsl]
        m['wd'] = _pack_lhsT(np.ascontiguousarray(Wd.T))
        # rank 0 carries the residual stream into the final output; the
        # host sums the TP group's partial outputs (no MLP collective)
        m['resw'] = np.full((128, 1), 1.0 if r == 0 else 0.0, np.float32)
        per_rank.append(m)

    return shared, per_rank, sa_cls, ca_cls


# ---------------------------------------------------------------- builder
def _build(sa_cls, ca_cls, nb_sa, nb_ca):
    nc = bacc.Bacc("TRN2", target_bir_lowering=False, debug=False,
                   num_devices=NCORES)

    d_x = nc.declare_dram_parameter("x", [C, S], BF, isOutput=False)
    d_tab = {k: nc.declare_dram_parameter(k, [HD, S], BF, isOutput=False)
             for k in ('sinq', 'cosq', 'sink', 'cosk')}
    d_omb = nc.declare_dram_parameter("ones_mat_bf", [128, 128], BF, isOutput=False)
    d_pr = nc.declare_dram_parameter("protT", [128, 128], BF, isOutput=False)
    d_msa = nc.declare_dram_parameter("mask_sa", [nb_sa, 128, 512], BF, isOutput=False)
    d_mca = nc.declare_dram_parameter("mask_ca", [nb_ca, 128, 512], BF, isOutput=False)
    d_w = {}
    for t in ('sa', 'ca'):
        for p in ('q', 'k'):
            d_w[f'w{p}_{t}'] = nc.declare_dram_parameter(
                f'w{p}_{t}', [OCA // 128, 128, C], BF, isOutput=False)
        d_w[f'wvT_{t}'] = nc.declare_dram_parameter(
            f'wvT_{t}', [CT, 128, OCA], BF, isOutput=False)
        d_w[f'wo_{t}'] = nc.declare_dram_parameter(
            f'wo_{t}', [CT, 128, OCA], BF, isOutput=False)
    for k in ('wg', 'wu', 'wd'):
        kdim = C if k != 'wd' else FFC
        d_w[k] = nc.declare_dram_parameter(k, [FFT, 128, kdim], BF, isOutput=False)
    d_resw = nc.declare_dram_parameter("resw", [128, 1], F32, isOutput=False)
    d_out = nc.declare_dram_parameter("out", [C, S], F32, isOutput=True)

    with tile.TileContext(nc) as tc:
        with (
            tc.tile_pool(name="const", bufs=1) as cpool,
            tc.tile_pool(name="xp", bufs=1) as xpool,
            tc.tile_pool(name="hp", bufs=1) as hpool,
            tc.tile_pool(name="wb", bufs=6) as wpool,
            tc.tile_pool(name="oo", bufs=3) as opool,
            tc.tile_pool(name="sm", bufs=2) as spool,
            tc.tile_pool(name="dram", bufs=1, space="DRAM") as dpool,
            tc.tile_pool(name="psA", bufs=7, space="PSUM") as psA,
        ):
            # ---------------- constants / tables ----------------
            def ptile(pool, shape, dt, name):
                return pool.tile(shape, dt, name=name, tag=name)

            xt = [ptile(xpool, [128, S], BF, f"x{cc}") for cc in range(CT)]
            for cc in range(CT):
                nc.sync.dma_start(xt[cc][:], d_x.ap()[cc * 128:(cc + 1) * 128, :])

            ones_mat_bf = ptile(cpool, [128, 128], BF, "ones_mat_bf")
            protT = ptile(cpool, [128, 128], BF, "protT")
            eps_t = ptile(cpool, [128, 1], F32, "eps_t")
            resw = ptile(cpool, [128, 1], F32, "resw")
            nc.sync.dma_start(ones_mat_bf[:], d_omb.ap())
            nc.sync.dma_start(protT[:], d_pr.ap())
            nc.sync.dma_start(resw[:], d_resw.ap())
            nc.vector.memset(eps_t[:], EPS)
            tabs = {}
            for k in d_tab:
                tabs[k] = ptile(cpool, [HD, S], BF, f"tab_{k}")
                nc.sync.dma_start(tabs[k][:], d_tab[k].ap())
            used_sa = {c for row in sa_cls for c in row if isinstance(c, int)}
            used_ca = {c for row in ca_cls for c in row if isinstance(c, int)}
            msk_sa, msk_ca = {}, {}
            for i in sorted(used_sa):
                msk_sa[i] = ptile(cpool, [128, 512], BF, f"msa{i}")
                nc.sync.dma_start(msk_sa[i][:], d_msa.ap()[i])
            for i in sorted(used_ca):
                msk_ca[i] = ptile(cpool, [128, 512], BF, f"mca{i}")
                nc.sync.dma_start(msk_ca[i][:], d_mca.ap()[i])

            # ---------------- residual stream x ----------------
            ht = [ptile(hpool, [128, S], BF, f"h{cc}") for cc in range(CT)]

            # ---------------- helpers ----------------
            def norm_sc(sc, scope):
                """ht[:, s0] = xt[:, s0] * rsqrt(mean_c(xt^2) + eps).
                Sum over C and partition-broadcast fused into one f32r
                matmul chain with an all-ones stationary."""
                s0 = slice(sc * 512, (sc + 1) * 512)
                with nc.named_scope(scope):
                    ss = psA.tile([128, 512], F32, tag="acc")
                    for cc in range(CT):
                        sq = spool.tile([128, 512], BF, tag="sq")
                        nc.scalar.activation(sq[:], xt[cc][:, s0], AF.Square)
                        nc.tensor.matmul(ss[:], ones_mat_bf[:], sq[:],
                                         start=(cc == 0), stop=(cc == CT - 1))
                    rs = spool.tile([128, 512], F32, tag="rs")
                    nc.scalar.activation(rs[:], ss[:], AF.Sqrt,
                                         bias=eps_t[:], scale=1.0 / C)
                    rr = spool.tile([128, 512], F32, tag="rr")
                    nc.vector.reciprocal_approx_fast(rr[:], rs[:])
                    for cc in range(CT):
                        nc.vector.tensor_tensor(ht[cc][:, s0], xt[cc][:, s0],
                                                rr[:], op=MULT)

            def res_sc(b_half, sc, scope, final=False):
                """xt[:, s0] += AR half (bf16 dram (C,512)); final -> write out."""
                s0 = slice(sc * 512, (sc + 1) * 512)
                with nc.named_scope(scope):
                    for cc in range(CT):
                        ar = opool.tile([128, 512], BF, tag="ar")
                        nc.sync.dma_start(ar[:], b_half[cc * 128:(cc + 1) * 128, :])
                        if final:
                            ot = opool.tile([128, 512], F32, tag="obuf")
                            nc.vector.tensor_tensor(ot[:], xt[cc][:, s0], ar[:],
                                                    op=ADD)
                            nc.sync.dma_start(
                                d_out.ap()[cc * 128:(cc + 1) * 128, s0], ot[:])
                        else:
                            nc.vector.tensor_tensor(xt[cc][:, s0], xt[cc][:, s0],
                                                    ar[:], op=ADD)

            def attention(t, cls, msk, apool, b_prev):
                """One attention block. b_prev: previous block's AR halves (or
                None); its residual is applied lazily per s-chunk here so the
                previous AllReduce overlaps this block's per-chunk compute.
                Returns this block's AR output halves."""
                qk_rope, vTc = {}, {}
                att = [apool.tile([128, S], BF, name=f"att{t}{oc}",
                                  tag=f"att{oc}", bufs=1) for oc in range(HPC)]
                b_in = [dpool.tile([C, 512], BF, name=f"bin_{t}{h}",
                                   tag=f"bin_{t}{h}") for h in range(ST)]
                b_out = [dpool.tile([C, 512], BF, name=f"bout_{t}{h}",
                                    tag=f"bout_{t}{h}") for h in range(ST)]
                # per-block weight preloads (wo + wvT, reused across halves);
                # DMAs emitted lazily at first use point to keep the queue
                # order aligned with consumption order.
                wvt, wot = {}, {}

                def load_wvt():
                    for cc in range(CT):
                        wvt[cc] = apool.tile([128, OCA], BF, name=f"wvT{t}{cc}",
                                             tag=f"wvT{cc}", bufs=1)
                        nc.sync.dma_start(wvt[cc][:], d_w[f'wvT_{t}'].ap()[cc])

                def load_wot():
                    for cc in range(CT):
                        wot[cc] = apool.tile([128, OCA], BF, name=f"wo{t}{cc}",
                                             tag=f"wo{cc}", bufs=1)
                        nc.sync.dma_start(wot[cc][:], d_w[f'wo_{t}'].ap()[cc])
                # per q-half: attention core, then immediately wo + AllReduce
                # for that s-half so the collective overlaps the other half's
                # attention (engine instruction streams are static - emission
                # order is execution order per engine).
                def attn_wo_qc(qc):
                    if not wot:
                        load_wot()
                    s0 = slice(qc * 512, (qc + 1) * 512)
                    with nc.named_scope(f"{t}_attn"):
                        for oc in range(HPC):
                            qr, kr = qk_rope[('q', oc)], qk_rope[('k', oc)]
                            valid = [kc for kc in range(KT) if cls[qc][kc] != 's']
                            probs = {}
                            for kc in valid:
                                sp = psA.tile([128, 512], F32, tag="acc")
                                nc.tensor.matmul(
                                    sp[:], kr[:, kc * 128:(kc + 1) * 128],
                                    qr[:, s0], start=True, stop=True)
                                pt = apool.tile([128, 512], BF, tag="probs",
                                                bufs=10)
                                nc.scalar.activation(pt[:], sp[:], AF.Exp)
                                if cls[qc][kc] != 'c':
                                    nc.vector.tensor_tensor(
                                        pt[:], pt[:], msk[cls[qc][kc]][:], op=MULT)
                                probs[kc] = pt
                            # denominator summed over k AND broadcast to all
                            # 128 partitions via the all-ones stationary
                            dnb = psA.tile([128, 512], F32, tag="acc")
                            for i, kc in enumerate(valid):
                                nc.tensor.matmul(dnb[:], ones_mat_bf[:],
                                                 probs[kc][:],
                                                 start=(i == 0),
                                                 stop=(i == len(valid) - 1))
                            rbs = spool.tile([128, 512], F32, tag="rbs")
                            nc.vector.reciprocal_approx_fast(rbs[:], dnb[:])
                            pa = psA.tile([128, 512], F32, tag="acc")
                            for i, kc in enumerate(valid):
                                nc.tensor.matmul(
                                    pa[:], vTc[kc][:, oc * 128:(oc + 1) * 128],
                                    probs[kc][:],
                                    start=(i == 0), stop=(i == len(valid) - 1))
                            nc.vector.tensor_tensor(att[oc][:, s0], pa[:], rbs[:],
                                                    op=MULT)
                    with nc.named_scope(f"{t}_wo"):
                        for cc in range(CT):
                            ps = psA.tile([128, 512], F32, tag="acc")
                            for ac in range(HPC):
                                nc.tensor.matmul(
                                    ps[:], wot[cc][:, ac * 128:(ac + 1) * 128],
                                    att[ac][:, s0],
                                    start=(ac == 0), stop=(ac == HPC - 1))
                            osb = opool.tile([128, 512], BF, tag="obuf")
                            nc.scalar.activation(osb[:], ps[:], AF.Copy)
                            nc.sync.dma_start(
                                b_in[qc][cc * 128:(cc + 1) * 128, :], osb[:])
                        nc.gpsimd.collective_compute(
                            "AllReduce", ADD, replica_groups=REPLICA_GROUPS,
                            ins=[b_in[qc][:].opt()], outs=[b_out[qc][:].opt()])

                # causal early path: if every non-skip key tile for q-half 0
                # lies in s-half 0, its attention + wo + AllReduce can be
                # emitted before s-half 1's projections exist.
                early = all(kc < KT // 2 for kc in range(KT)
                            if cls[0][kc] != 's')
                for sc in range(ST):
                    s0 = slice(sc * 512, (sc + 1) * 512)
                    if b_prev is not None:
                        # scheduler-only fence: keep every engine's queue
                        # order aligned with emission order here, so ops
                        # depending on the previous block's AllReduce can't
                        # be hoisted ahead of this block's independent work
                        # (head-of-line blocking on the strict-FIFO queues).
                        tc.no_sync_barrier()
                        res_sc(b_prev[sc], sc, f"{t}_res")
                    norm_sc(sc, f"{t}_norm")
                    with nc.named_scope(f"{t}_qkv"):
                        for oc in range(HPC):
                            if ('k', oc) not in qk_rope:
                                qk_rope[('k', oc)] = apool.tile(
                                    [128, S], BF, name=f"kr{t}{oc}",
                                    tag=f"kr{oc}", bufs=1)
                            wsb = wpool.tile([128, C], BF, tag="wbig")
                            nc.sync.dma_start(wsb[:], d_w[f'wk_{t}'].ap()[oc])
                            ps = psA.tile([128, 512], F32, tag="acc")
                            for cc in range(CT):
                                nc.tensor.matmul(
                                    ps[:], wsb[:, cc * 128:(cc + 1) * 128],
                                    ht[cc][:, s0],
                                    start=(cc == 0), stop=(cc == CT - 1))
                            lin = spool.tile([128, 512], BF, tag="lin")
                            nc.scalar.activation(lin[:], ps[:], AF.Copy)
                            rot = psA.tile([128, 512], F32, tag="acc")
                            nc.tensor.matmul(rot[:], protT[:], lin[:],
                                             start=True, stop=True)
                            dst = qk_rope[('k', oc)]
                            nc.vector.tensor_tensor(
                                dst[:, s0], lin[:], tabs['cosk'][:, s0], op=MULT)
                            s2 = spool.tile([128, 512], BF, tag="rsc")
                            nc.vector.tensor_tensor(
                                s2[:], rot[:], tabs['sink'][:, s0], op=MULT)
                            nc.vector.tensor_tensor(
                                dst[:, s0], dst[:, s0], s2[:], op=ADD)
                        # V directly in [k-pos, oc*HD] layout: stationary =
                        # normalized-activation chunk, moving = WvT chunk
                        if not wvt:
                            load_wvt()
                        for j in range(4):
                            kc = sc * 4 + j
                            vTc[kc] = apool.tile([128, OCA], BF,
                                                 name=f"vT{t}{kc}",
                                                 tag=f"vT{kc}", bufs=1)
                            ps = psA.tile([128, 512], F32, tag="acc")
                            sblk = slice(sc * 512 + j * 128,
                                         sc * 512 + (j + 1) * 128)
                            for cc in range(CT):
                                nc.tensor.matmul(ps[:], ht[cc][:, sblk],
                                                 wvt[cc][:],
                                                 start=(cc == 0),
                                                 stop=(cc == CT - 1))
                            nc.scalar.activation(vTc[kc][:], ps[:], AF.Copy)
                        for oc in range(HPC):
                            if ('q', oc) not in qk_rope:
                                qk_rope[('q', oc)] = apool.tile(
                                    [128, S], BF, name=f"qr{t}{oc}",
                                    tag=f"qr{oc}", bufs=1)
                            wsb = wpool.tile([128, C], BF, tag="wbig")
                            nc.sync.dma_start(wsb[:], d_w[f'wq_{t}'].ap()[oc])
                            ps = psA.tile([128, 512], F32, tag="acc")
                            for cc in range(CT):
                                nc.tensor.matmul(
                                    ps[:], wsb[:, cc * 128:(cc + 1) * 128],
                                    ht[cc][:, s0],
                                    start=(cc == 0), stop=(cc == CT - 1))
                            lin = spool.tile([128, 512], BF, tag="lin")
                            nc.scalar.activation(lin[:], ps[:], AF.Copy)
                            rot = psA.tile([128, 512], F32, tag="acc")
                            nc.tensor.matmul(rot[:], protT[:], lin[:],
                                             start=True, stop=True)
                            dst = qk_rope[('q', oc)]
                            nc.vector.tensor_tensor(
                                dst[:, s0], lin[:], tabs['cosq'][:, s0], op=MULT)
                            s2 = spool.tile([128, 512], BF, tag="rsc")
                            nc.vector.tensor_tensor(
                                s2[:], rot[:], tabs['sinq'][:, s0], op=MULT)
                            nc.vector.tensor_tensor(
                                dst[:, s0], dst[:, s0], s2[:], op=ADD)
                    if sc == 0 and early:
                        attn_wo_qc(0)
                for qc in range(ST):
                    if qc == 0 and early:
                        continue
                    attn_wo_qc(qc)
                return b_out

            # ================= attention blocks =================
            with tc.tile_pool(name="ap", bufs=1) as apool:
                b_sa = attention('sa', sa_cls, msk_sa, apool, None)
                b_ca = attention('ca', ca_cls, msk_ca, apool, b_sa)

            # ================= MLP =================
            # No MLP collectives: each rank writes its partial down-proj sum
            # (plus the replicated residual, weighted by resw so only rank 0
            # contributes it once) to its own output; the host sums the TP
            # group's outputs.
            mpool_ctx = tc.tile_pool(name="mp", bufs=1)
            mpool = mpool_ctx.__enter__()
            gact = [mpool.tile([128, S], BF, name=f"gact{f}", tag=f"gact{f}",
                               bufs=1) for f in range(FFT)]

            def mlp_up(sc):
                s0 = slice(sc * 512, (sc + 1) * 512)
                with nc.named_scope("mlp_up"):
                    for f in range(FFT):
                        wg = wpool.tile([128, C], BF, tag="wbig")
                        nc.sync.dma_start(wg[:], d_w['wg'].ap()[f])
                        wu = wpool.tile([128, C], BF, tag="wbig")
                        nc.sync.dma_start(wu[:], d_w['wu'].ap()[f])
                        pg = psA.tile([128, 512], F32, tag="acc")
                        for cc in range(CT):
                            nc.tensor.matmul(pg[:], wg[:, cc * 128:(cc + 1) * 128],
                                             ht[cc][:, s0],
                                             start=(cc == 0), stop=(cc == CT - 1))
                        pu = psA.tile([128, 512], F32, tag="acc")
                        for cc in range(CT):
                            nc.tensor.matmul(pu[:], wu[:, cc * 128:(cc + 1) * 128],
                                             ht[cc][:, s0],
                                             start=(cc == 0), stop=(cc == CT - 1))
                        gs = spool.tile([128, 512], BF, tag="lin")
                        nc.scalar.activation(gs[:], pg[:], AF.Silu)
                        nc.vector.tensor_tensor(gact[f][:, s0], gs[:], pu[:],
                                                op=MULT)

            def mlp_down_out(sc):
                s0 = slice(sc * 512, (sc + 1) * 512)
                with nc.named_scope("mlp_down"):
                    for cc in range(CT):
                        wd = wpool.tile([128, FFC], BF, tag="wbig")
                        nc.sync.dma_start(wd[:], d_w['wd'].ap()[cc])
                        ps = psA.tile([128, 512], F32, tag="acc")
                        for f in range(FFT):
                            nc.tensor.matmul(ps[:], wd[:, f * 128:(f + 1) * 128],
                                             gact[f][:, s0],
                                             start=(f == 0), stop=(f == FFT - 1))
                        ot = opool.tile([128, 512], F32, tag="fout")
                        nc.vector.tensor_scalar(ot[:], xt[cc][:, s0], resw[:],
                                                None, op0=MULT)
                        nc.vector.tensor_tensor(ot[:], ot[:], ps[:], op=ADD)
                        nc.sync.dma_start(
                            d_out.ap()[cc * 128:(cc + 1) * 128, s0], ot[:])

            tc.no_sync_barrier()
            res_sc(b_ca[0], 0, "ca_res")
            norm_sc(0, "mlp_norm")
            mlp_up(0)
            mlp_down_out(0)
            tc.no_sync_barrier()
            res_sc(b_ca[1], 1, "ca_res")
            norm_sc(1, "mlp_norm")
            mlp_up(1)
            mlp_down_out(1)
            mpool_ctx.__exit__(None, None, None)

    nc.compile()
    return nc


# ---------------------------------------------------------------- entry
def _mask_sig(cls, pat):
    return (tuple(tuple(row) for row in cls), pat.tobytes())


def kernel(**inputs) -> np.ndarray:
    shared, per_rank, sa_cls, ca_cls = _prep_host(inputs)
    nb_sa, nb_ca = shared['mask_sa'].shape[0], shared['mask_ca'].shape[0]

    key = (_mask_sig(sa_cls, shared['mask_sa']),
           _mask_sig(ca_cls, shared['mask_ca']))
    if key not in _CACHE:
        _CACHE[key] = _build(sa_cls, ca_cls, nb_sa, nb_ca)
    nc = _CACHE[key]

    x = np.asarray(inputs['x'], dtype=np.float32)
    xb = [np.ascontiguousarray(x[g]).astype(BF_NP) for g in range(B)]
    in_maps = []
    for core in range(NCORES):
        g, r = core // TPG, core % TPG
        m = dict(shared)
        m['x'] = xb[g]
        m.update(per_rank[r])
        in_maps.append(m)

    res = run_bass_kernel_spmd(nc, in_maps, core_ids=list(range(NCORES)))
    outs = []
    for g in range(B):
        acc = np.asarray(res.results[g * TPG]['out'], dtype=np.float32).copy()
        for r in range(1, TPG):
            acc += np.asarray(res.results[g * TPG + r]['out'], dtype=np.float32)
        outs.append(acc)
    return np.stack(outs, axis=0)


# revision 19
# speedup vs baseline: 1.2148x; 1.0209x over previous
"""Self-contained Trainium2 Bass kernel for nn_ANEDecoderLayer (ANE decoder layer).

Shapes (hardcoded): B=2, C=2048, S=1024, H=16, HD=128, FF=8192, fp32 I/O.

Sharding: hybrid batch(2) x tensor-parallel(4) over 8 NeuronCores.
  core = g*4 + r:  g = batch index, r = TP rank.
  Within each group of 4 cores: heads sharded 4/core, d_ff sharded 2048/core.
  Block outputs (row-split Wo / W_down partial sums) are AllReduced in bf16
  within each 4-core group: replica_groups [[0,1,2,3],[4,5,6,7]].

Host-side preprocessing:
  - RMSNorm weights folded into the following matmul weights.
  - Weights pre-transposed, pre-tiled for lhsT layout, cast to bf16.
  - sin_k/cos_k pre-scaled by 1/sqrt(HD) (folds attention scale into K).
  - kv cache scatter (kv_write_idx) folded into a row-permutation of the mask.
  - additive masks converted to multiplicative exp(mask) tile patterns with
    per-tile classification (all-ones -> no op, all-zero -> tile skipped,
    else multiply by a deduplicated pattern tile).

Compute: matmuls in bf16 (fp32 PSUM accumulation); RMSNorm statistics via an
all-ones stationary matmul in float32r (sum + partition-broadcast fused, 1
cycle/row); softmax denominators likewise summed+broadcast with an all-ones
bf16 stationary; V is produced directly in [k-position, head*HD] layout by
making the normalized-activation chunk the stationary operand (no PE
transposes); softmax without max-subtraction (scores are O(5); exp(-1e9)=0
handled by tile skipping); residual stream held in bf16 in SBUF.
"""

import numpy as np
import ml_dtypes

import concourse.mybir as mybir
import concourse.tile as tile
from concourse import bacc
from concourse.bass_utils import run_bass_kernel_spmd

# ---------------------------------------------------------------- constants
B, C, S, H, HD, FF = 2, 2048, 1024, 16, 128, 8192
EPS = 1e-5
SCALE = 1.0 / float(np.sqrt(HD))

NCORES = 8
TPG = 4                      # tensor-parallel group size
HPC = H // TPG               # heads per core = 4
OCA = HPC * HD               # attention out-channels per core = 512
FFC = FF // TPG              # ff channels per core = 2048

CT = C // 128                # 16 c-chunks
ST = S // 512                # 2 s-chunks of 512
KT = S // 128                # 8 k-chunks
FFT = FFC // 128             # 16 ff-chunks per core

F32 = mybir.dt.float32
F32R = mybir.dt.float32r
BF = mybir.dt.bfloat16
AF = mybir.ActivationFunctionType
MULT = mybir.AluOpType.mult
ADD = mybir.AluOpType.add
BF_NP = ml_dtypes.bfloat16

REPLICA_GROUPS = [[0, 1, 2, 3], [4, 5, 6, 7]]

_CACHE: dict = {}


# ---------------------------------------------------------------- host prep
def _pack_lhsT(wT: np.ndarray) -> np.ndarray:
    """wT: (K, M) contraction-major weight. Returns (M//128, 128, K) bf16 where
    pack[m][p, kc*128+f] = wT[kc*128+p, m*128+f]; a DMA of pack[m] gives an
    SBUF tile whose slice [:, kc*128:(kc+1)*128] is the lhsT for contraction
    chunk kc -> output chunk m."""
    K, M = wT.shape
    Kt, Mt = K // 128, M // 128
    t = wT.reshape(Kt, 128, Mt, 128)              # [kc, p, m, f]
    t = t.transpose(2, 1, 0, 3).reshape(Mt, 128, K)
    return np.ascontiguousarray(t.astype(BF_NP))


def _classify_mask(mask_eff: np.ndarray):
    """mask_eff: (S, S) additive mask, (k, q) orientation. Returns
    (cls, patterns): cls[qc][kc] in {'c' (clean), 's' (skip), int idx};
    patterns: (NB, 128, 512) bf16 multiplicative tiles."""
    mm = np.exp(np.minimum(mask_eff.astype(np.float64), 0.0)).astype(np.float32)
    # positive masks would overflow exp; reference masks are <= 0
    if mask_eff.max() > 0:
        mm = np.exp(mask_eff.astype(np.float64)).astype(np.float32)
    patterns = []
    keys = {}
    cls = [[None] * KT for _ in range(ST)]
    for qc in range(ST):
        for kc in range(KT):
            sub = mm[kc * 128:(kc + 1) * 128, qc * 512:(qc + 1) * 512]
            if np.all(sub == 1.0):
                cls[qc][kc] = 'c'
            elif np.all(sub == 0.0):
                cls[qc][kc] = 's'
            else:
                kb = sub.tobytes()
                if kb not in keys:
                    keys[kb] = len(patterns)
                    patterns.append(sub.astype(BF_NP))
                cls[qc][kc] = keys[kb]
    if patterns:
        pat = np.stack(patterns)
    else:
        pat = np.zeros((1, 128, 512), BF_NP)
    return cls, pat


def _prep_host(inputs):
    """Returns (shared_map, per_rank_maps, sa_cls, ca_cls)."""
    g = lambda k: np.asarray(inputs[k], dtype=np.float32)

    sinq = np.ascontiguousarray(g('sin_q').reshape(HD, S))
    cosq = np.ascontiguousarray(g('cos_q').reshape(HD, S))
    sink = np.ascontiguousarray(g('sin_k').reshape(HD, S) * SCALE)
    cosk = np.ascontiguousarray(g('cos_k').reshape(HD, S) * SCALE)

    idx = np.asarray(inputs['kv_write_idx']).astype(np.int64)
    if not np.array_equal(np.sort(idx), np.arange(S)):
        raise NotImplementedError("kv_write_idx must be a permutation of arange(S)")
    sa_mask = g('self_attn_mask').reshape(S, S)[idx, :]     # effective (k, q) mask
    ca_mask = g('cross_attn_mask').reshape(S, S)
    sa_cls, sa_pat = _classify_mask(sa_mask)
    ca_cls, ca_pat = _classify_mask(ca_mask)

    P_rot = np.zeros((HD, HD), np.float32)
    P_rot[np.arange(64), np.arange(64, 128)] = -1.0
    P_rot[np.arange(64, 128), np.arange(64)] = 1.0

    shared = {
        'sinq': sinq.astype(BF_NP), 'cosq': cosq.astype(BF_NP),
        'sink': sink.astype(BF_NP), 'cosk': cosk.astype(BF_NP),
        'ones_mat_bf': np.ones((128, 128), BF_NP),
        'protT': np.ascontiguousarray(P_rot.T).astype(BF_NP),
        'mask_sa': sa_pat, 'mask_ca': ca_pat,
    }

    w_sa, w_ca, w_mlp = g('w_sa'), g('w_ca'), g('w_mlp')
    per_rank = []
    for r in range(TPG):
        asl = slice(r * OCA, (r + 1) * OCA)
        fsl = slice(r * FFC, (r + 1) * FFC)
        m = {}
        for tag in ('sa', 'ca'):
            wnorm = w_sa if tag == 'sa' else w_ca
            for p in ('q', 'k'):
                W = g(f'w{p}_{tag}')[asl, :] * wnorm[None, :]
                m[f'w{p}_{tag}'] = _pack_lhsT(np.ascontiguousarray(W.T))
            Wv = g(f'wv_{tag}')[asl, :] * wnorm[None, :]
            # moving-operand layout: (CT, 128, OCA); partition = c chunk
            m[f'wvT_{tag}'] = np.ascontiguousarray(
                Wv.T.reshape(CT, 128, OCA).astype(BF_NP))
            Wo = g(f'wo_{tag}')[:, asl]
            m[f'wo_{tag}'] = _pack_lhsT(np.ascontiguousarray(Wo.T))
        for p, key in (('g', 'w_gate'), ('u', 'w_up')):
            W = g(key)[fsl, :] * w_mlp[None, :]
            m[f'w{p}'] = _pack_lhsT(np.ascontiguousarray(W.T))
        Wd = g('w_down')[:, fsl]
        m['wd'] = _pack_lhsT(np.ascontiguousarray(Wd.T))
        # rank 0 carries the residual stream into the final output; the
        # host sums the TP group's partial outputs (no MLP collective)
        m['resw'] = np.full((128, 1), 1.0 if r == 0 else 0.0, np.float32)
        per_rank.append(m)

    return shared, per_rank, sa_cls, ca_cls


# ---------------------------------------------------------------- builder
def _build(sa_cls, ca_cls, nb_sa, nb_ca):
    nc = bacc.Bacc("TRN2", target_bir_lowering=False, debug=False,
                   num_devices=NCORES)

    d_x = nc.declare_dram_parameter("x", [C, S], BF, isOutput=False)
    d_tab = {k: nc.declare_dram_parameter(k, [HD, S], BF, isOutput=False)
             for k in ('sinq', 'cosq', 'sink', 'cosk')}
    d_omb = nc.declare_dram_parameter("ones_mat_bf", [128, 128], BF, isOutput=False)
    d_pr = nc.declare_dram_parameter("protT", [128, 128], BF, isOutput=False)
    d_msa = nc.declare_dram_parameter("mask_sa", [nb_sa, 128, 512], BF, isOutput=False)
    d_mca = nc.declare_dram_parameter("mask_ca", [nb_ca, 128, 512], BF, isOutput=False)
    d_w = {}
    for t in ('sa', 'ca'):
        for p in ('q', 'k'):
            d_w[f'w{p}_{t}'] = nc.declare_dram_parameter(
                f'w{p}_{t}', [OCA // 128, 128, C], BF, isOutput=False)
        d_w[f'wvT_{t}'] = nc.declare_dram_parameter(
            f'wvT_{t}', [CT, 128, OCA], BF, isOutput=False)
        d_w[f'wo_{t}'] = nc.declare_dram_parameter(
            f'wo_{t}', [CT, 128, OCA], BF, isOutput=False)
    for k in ('wg', 'wu', 'wd'):
        kdim = C if k != 'wd' else FFC
        d_w[k] = nc.declare_dram_parameter(k, [FFT, 128, kdim], BF, isOutput=False)
    d_resw = nc.declare_dram_parameter("resw", [128, 1], F32, isOutput=False)
    d_out = nc.declare_dram_parameter("out", [C, S], F32, isOutput=True)

    with tile.TileContext(nc) as tc:
        with (
            tc.tile_pool(name="const", bufs=1) as cpool,
            tc.tile_pool(name="xp", bufs=1) as xpool,
            tc.tile_pool(name="hp", bufs=1) as hpool,
            tc.tile_pool(name="wb", bufs=6) as wpool,
            tc.tile_pool(name="oo", bufs=3) as opool,
            tc.tile_pool(name="sm", bufs=2) as spool,
            tc.tile_pool(name="dram", bufs=1, space="DRAM") as dpool,
            tc.tile_pool(name="psA", bufs=7, space="PSUM") as psA,
        ):
            # ---------------- constants / tables ----------------
            def ptile(pool, shape, dt, name):
                return pool.tile(shape, dt, name=name, tag=name)

            xt = [ptile(xpool, [128, S], BF, f"x{cc}") for cc in range(CT)]
            for cc in range(CT):
                nc.sync.dma_start(xt[cc][:], d_x.ap()[cc * 128:(cc + 1) * 128, :])

            ones_mat_bf = ptile(cpool, [128, 128], BF, "ones_mat_bf")
            protT = ptile(cpool, [128, 128], BF, "protT")
            eps_t = ptile(cpool, [128, 1], F32, "eps_t")
            resw = ptile(cpool, [128, 1], F32, "resw")
            nc.sync.dma_start(ones_mat_bf[:], d_omb.ap())
            nc.sync.dma_start(protT[:], d_pr.ap())
            nc.sync.dma_start(resw[:], d_resw.ap())
            nc.vector.memset(eps_t[:], EPS)
            tabs = {}
            for k in d_tab:
                tabs[k] = ptile(cpool, [HD, S], BF, f"tab_{k}")
                nc.sync.dma_start(tabs[k][:], d_tab[k].ap())
            used_sa = {c for row in sa_cls for c in row if isinstance(c, int)}
            used_ca = {c for row in ca_cls for c in row if isinstance(c, int)}
            msk_sa, msk_ca = {}, {}
            for i in sorted(used_sa):
                msk_sa[i] = ptile(cpool, [128, 512], BF, f"msa{i}")
                nc.sync.dma_start(msk_sa[i][:], d_msa.ap()[i])
            for i in sorted(used_ca):
                msk_ca[i] = ptile(cpool, [128, 512], BF, f"mca{i}")
                nc.sync.dma_start(msk_ca[i][:], d_mca.ap()[i])

            # ---------------- residual stream x ----------------
            ht = [ptile(hpool, [128, S], BF, f"h{cc}") for cc in range(CT)]

            # ---------------- helpers ----------------
            def norm_sc(sc, scope):
                """ht[:, s0] = xt[:, s0] * rsqrt(mean_c(xt^2) + eps).
                Sum over C and partition-broadcast fused into one f32r
                matmul chain with an all-ones stationary."""
                s0 = slice(sc * 512, (sc + 1) * 512)
                with nc.named_scope(scope):
                    ss = psA.tile([128, 512], F32, tag="acc")
                    for cc in range(CT):
                        sq = spool.tile([128, 512], BF, tag="sq")
                        nc.scalar.activation(sq[:], xt[cc][:, s0], AF.Square)
                        nc.tensor.matmul(ss[:], ones_mat_bf[:], sq[:],
                                         start=(cc == 0), stop=(cc == CT - 1))
                    rs = spool.tile([128, 512], F32, tag="rs")
                    nc.scalar.activation(rs[:], ss[:], AF.Sqrt,
                                         bias=eps_t[:], scale=1.0 / C)
                    rr = spool.tile([128, 512], F32, tag="rr")
                    nc.vector.reciprocal_approx_fast(rr[:], rs[:])
                    for cc in range(CT):
                        nc.vector.tensor_tensor(ht[cc][:, s0], xt[cc][:, s0],
                                                rr[:], op=MULT)

            def res_sc(b_half, sc, scope, final=False):
                """xt[:, s0] += AR half (bf16 dram (C,512)); final -> write out."""
                s0 = slice(sc * 512, (sc + 1) * 512)
                with nc.named_scope(scope):
                    for cc in range(CT):
                        ar = opool.tile([128, 512], BF, tag="ar")
                        # Activation-engine HWDGE queue: keeps these reads off
                        # the weight-stream queue (no queuing behind prefetches)
                        nc.scalar.dma_start(ar[:], b_half[cc * 128:(cc + 1) * 128, :])
                        if final:
                            ot = opool.tile([128, 512], F32, tag="obuf")
                            nc.vector.tensor_tensor(ot[:], xt[cc][:, s0], ar[:],
                                                    op=ADD)
                            nc.sync.dma_start(
                                d_out.ap()[cc * 128:(cc + 1) * 128, s0], ot[:])
                        else:
                            nc.vector.tensor_tensor(xt[cc][:, s0], xt[cc][:, s0],
                                                    ar[:], op=ADD)

            def attention(t, cls, msk, apool, b_prev):
                """One attention block. b_prev: previous block's AR halves (or
                None); its residual is applied lazily per s-chunk here so the
                previous AllReduce overlaps this block's per-chunk compute.
                Returns this block's AR output halves."""
                qk_rope, vTc = {}, {}
                att = [apool.tile([128, S], BF, name=f"att{t}{oc}",
                                  tag=f"att{oc}", bufs=1) for oc in range(HPC)]
                b_in = [dpool.tile([C, 512], BF, name=f"bin_{t}{h}",
                                   tag=f"bin_{t}{h}") for h in range(ST)]
                b_out = [dpool.tile([C, 512], BF, name=f"bout_{t}{h}",
                                    tag=f"bout_{t}{h}") for h in range(ST)]
                # per-block weight preloads (wo + wvT, reused across halves);
                # DMAs emitted lazily at first use point to keep the queue
                # order aligned with consumption order.
                wvt, wot = {}, {}

                def load_wvt():
                    for cc in range(CT):
                        wvt[cc] = apool.tile([128, OCA], BF, name=f"wvT{t}{cc}",
                                             tag=f"wvT{cc}", bufs=1)
                        nc.sync.dma_start(wvt[cc][:], d_w[f'wvT_{t}'].ap()[cc])

                def load_wot():
                    for cc in range(CT):
                        wot[cc] = apool.tile([128, OCA], BF, name=f"wo{t}{cc}",
                                             tag=f"wo{cc}", bufs=1)
                        nc.sync.dma_start(wot[cc][:], d_w[f'wo_{t}'].ap()[cc])
                # per q-half: attention core, then immediately wo + AllReduce
                # for that s-half so the collective overlaps the other half's
                # attention (engine instruction streams are static - emission
                # order is execution order per engine).
                def attn_wo_qc(qc):
                    if not wot:
                        load_wot()
                    s0 = slice(qc * 512, (qc + 1) * 512)
                    with nc.named_scope(f"{t}_attn"):
                        for oc in range(HPC):
                            qr, kr = qk_rope[('q', oc)], qk_rope[('k', oc)]
                            valid = [kc for kc in range(KT) if cls[qc][kc] != 's']
                            probs = {}
                            for kc in valid:
                                sp = psA.tile([128, 512], F32, tag="acc")
                                nc.tensor.matmul(
                                    sp[:], kr[:, kc * 128:(kc + 1) * 128],
                                    qr[:, s0], start=True, stop=True)
                                pt = apool.tile([128, 512], BF, tag="probs",
                                                bufs=10)
                                nc.scalar.activation(pt[:], sp[:], AF.Exp)
                                if cls[qc][kc] != 'c':
                                    nc.vector.tensor_tensor(
                                        pt[:], pt[:], msk[cls[qc][kc]][:], op=MULT)
                                probs[kc] = pt
                            # denominator summed over k AND broadcast to all
                            # 128 partitions via the all-ones stationary
                            dnb = psA.tile([128, 512], F32, tag="acc")
                            for i, kc in enumerate(valid):
                                nc.tensor.matmul(dnb[:], ones_mat_bf[:],
                                                 probs[kc][:],
                                                 start=(i == 0),
                                                 stop=(i == len(valid) - 1))
                            rbs = spool.tile([128, 512], F32, tag="rbs")
                            nc.vector.reciprocal_approx_fast(rbs[:], dnb[:])
                            pa = psA.tile([128, 512], F32, tag="acc")
                            for i, kc in enumerate(valid):
                                nc.tensor.matmul(
                                    pa[:], vTc[kc][:, oc * 128:(oc + 1) * 128],
                                    probs[kc][:],
                                    start=(i == 0), stop=(i == len(valid) - 1))
                            nc.vector.tensor_tensor(att[oc][:, s0], pa[:], rbs[:],
                                                    op=MULT)
                    with nc.named_scope(f"{t}_wo"):
                        for cc in range(CT):
                            ps = psA.tile([128, 512], F32, tag="acc")
                            for ac in range(HPC):
                                nc.tensor.matmul(
                                    ps[:], wot[cc][:, ac * 128:(ac + 1) * 128],
                                    att[ac][:, s0],
                                    start=(ac == 0), stop=(ac == HPC - 1))
                            osb = opool.tile([128, 512], BF, tag="obuf")
                            nc.scalar.activation(osb[:], ps[:], AF.Copy)
                            nc.scalar.dma_start(
                                b_in[qc][cc * 128:(cc + 1) * 128, :], osb[:])
                        nc.gpsimd.collective_compute(
                            "AllReduce", ADD, replica_groups=REPLICA_GROUPS,
                            ins=[b_in[qc][:].opt()], outs=[b_out[qc][:].opt()])

                # causal early path: if every non-skip key tile for q-half 0
                # lies in s-half 0, its attention + wo + AllReduce can be
                # emitted before s-half 1's projections exist.
                early = all(kc < KT // 2 for kc in range(KT)
                            if cls[0][kc] != 's')
                for sc in range(ST):
                    s0 = slice(sc * 512, (sc + 1) * 512)
                    if b_prev is not None:
                        # scheduler-only fence: keep every engine's queue
                        # order aligned with emission order here, so ops
                        # depending on the previous block's AllReduce can't
                        # be hoisted ahead of this block's independent work
                        # (head-of-line blocking on the strict-FIFO queues).
                        tc.no_sync_barrier()
                        res_sc(b_prev[sc], sc, f"{t}_res")
                    norm_sc(sc, f"{t}_norm")
                    with nc.named_scope(f"{t}_qkv"):
                        for oc in range(HPC):
                            if ('k', oc) not in qk_rope:
                                qk_rope[('k', oc)] = apool.tile(
                                    [128, S], BF, name=f"kr{t}{oc}",
                                    tag=f"kr{oc}", bufs=1)
                            wsb = wpool.tile([128, C], BF, tag="wbig")
                            nc.sync.dma_start(wsb[:], d_w[f'wk_{t}'].ap()[oc])
                            ps = psA.tile([128, 512], F32, tag="acc")
                            for cc in range(CT):
                                nc.tensor.matmul(
                                    ps[:], wsb[:, cc * 128:(cc + 1) * 128],
                                    ht[cc][:, s0],
                                    start=(cc == 0), stop=(cc == CT - 1))
                            lin = spool.tile([128, 512], BF, tag="lin")
                            nc.scalar.activation(lin[:], ps[:], AF.Copy)
                            rot = psA.tile([128, 512], F32, tag="acc")
                            nc.tensor.matmul(rot[:], protT[:], lin[:],
                                             start=True, stop=True)
                            dst = qk_rope[('k', oc)]
                            nc.vector.tensor_tensor(
                                dst[:, s0], lin[:], tabs['cosk'][:, s0], op=MULT)
                            s2 = spool.tile([128, 512], BF, tag="rsc")
                            nc.vector.tensor_tensor(
                                s2[:], rot[:], tabs['sink'][:, s0], op=MULT)
                            nc.vector.tensor_tensor(
                                dst[:, s0], dst[:, s0], s2[:], op=ADD)
                        # V directly in [k-pos, oc*HD] layout: stationary =
                        # normalized-activation chunk, moving = WvT chunk
                        if not wvt:
                            load_wvt()
                        for j in range(4):
                            kc = sc * 4 + j
                            vTc[kc] = apool.tile([128, OCA], BF,
                                                 name=f"vT{t}{kc}",
                                                 tag=f"vT{kc}", bufs=1)
                            ps = psA.tile([128, 512], F32, tag="acc")
                            sblk = slice(sc * 512 + j * 128,
                                         sc * 512 + (j + 1) * 128)
                            for cc in range(CT):
                                nc.tensor.matmul(ps[:], ht[cc][:, sblk],
                                                 wvt[cc][:],
                                                 start=(cc == 0),
                                                 stop=(cc == CT - 1))
                            nc.scalar.activation(vTc[kc][:], ps[:], AF.Copy)
                        # no causal early path: q-half 0 attends the full key
                        # range, so its attention+wo+AllReduce only needs k/v
                        # of this half plus q of half 0 — emit it before this
                        # half's q-projection to trigger the collective sooner
                        if sc == 1 and not early:
                            attn_wo_qc(0)
                        for oc in range(HPC):
                            if ('q', oc) not in qk_rope:
                                qk_rope[('q', oc)] = apool.tile(
                                    [128, S], BF, name=f"qr{t}{oc}",
                                    tag=f"qr{oc}", bufs=1)
                            wsb = wpool.tile([128, C], BF, tag="wbig")
                            nc.sync.dma_start(wsb[:], d_w[f'wq_{t}'].ap()[oc])
                            ps = psA.tile([128, 512], F32, tag="acc")
                            for cc in range(CT):
                                nc.tensor.matmul(
                                    ps[:], wsb[:, cc * 128:(cc + 1) * 128],
                                    ht[cc][:, s0],
                                    start=(cc == 0), stop=(cc == CT - 1))
                            lin = spool.tile([128, 512], BF, tag="lin")
                            nc.scalar.activation(lin[:], ps[:], AF.Copy)
                            rot = psA.tile([128, 512], F32, tag="acc")
                            nc.tensor.matmul(rot[:], protT[:], lin[:],
                                             start=True, stop=True)
                            dst = qk_rope[('q', oc)]
                            nc.vector.tensor_tensor(
                                dst[:, s0], lin[:], tabs['cosq'][:, s0], op=MULT)
                            s2 = spool.tile([128, 512], BF, tag="rsc")
                            nc.vector.tensor_tensor(
                                s2[:], rot[:], tabs['sinq'][:, s0], op=MULT)
                            nc.vector.tensor_tensor(
                                dst[:, s0], dst[:, s0], s2[:], op=ADD)
                    if sc == 0 and early:
                        attn_wo_qc(0)
                for qc in range(1, ST):
                    attn_wo_qc(qc)
                return b_out

            # ================= attention blocks =================
            with tc.tile_pool(name="ap", bufs=1) as apool:
                b_sa = attention('sa', sa_cls, msk_sa, apool, None)
                b_ca = attention('ca', ca_cls, msk_ca, apool, b_sa)

            # ================= MLP =================
            # No MLP collectives: each rank writes its partial down-proj sum
            # (plus the replicated residual, weighted by resw so only rank 0
            # contributes it once) to its own output; the host sums the TP
            # group's outputs.
            mpool_ctx = tc.tile_pool(name="mp", bufs=1)
            mpool = mpool_ctx.__enter__()
            gact = [mpool.tile([128, S], BF, name=f"gact{f}", tag=f"gact{f}",
                               bufs=1) for f in range(FFT)]

            def mlp_up(sc):
                s0 = slice(sc * 512, (sc + 1) * 512)
                with nc.named_scope("mlp_up"):
                    for f in range(FFT):
                        wg = wpool.tile([128, C], BF, tag="wbig")
                        nc.sync.dma_start(wg[:], d_w['wg'].ap()[f])
                        wu = wpool.tile([128, C], BF, tag="wbig")
                        nc.sync.dma_start(wu[:], d_w['wu'].ap()[f])
                        pg = psA.tile([128, 512], F32, tag="acc")
                        for cc in range(CT):
                            nc.tensor.matmul(pg[:], wg[:, cc * 128:(cc + 1) * 128],
                                             ht[cc][:, s0],
                                             start=(cc == 0), stop=(cc == CT - 1))
                        pu = psA.tile([128, 512], F32, tag="acc")
                        for cc in range(CT):
                            nc.tensor.matmul(pu[:], wu[:, cc * 128:(cc + 1) * 128],
                                             ht[cc][:, s0],
                                             start=(cc == 0), stop=(cc == CT - 1))
                        gs = spool.tile([128, 512], BF, tag="lin")
                        nc.scalar.activation(gs[:], pg[:], AF.Silu)
                        nc.vector.tensor_tensor(gact[f][:, s0], gs[:], pu[:],
                                                op=MULT)

            def mlp_down_out(sc):
                s0 = slice(sc * 512, (sc + 1) * 512)
                with nc.named_scope("mlp_down"):
                    for cc in range(CT):
                        wd = wpool.tile([128, FFC], BF, tag="wbig")
                        nc.sync.dma_start(wd[:], d_w['wd'].ap()[cc])
                        ps = psA.tile([128, 512], F32, tag="acc")
                        for f in range(FFT):
                            nc.tensor.matmul(ps[:], wd[:, f * 128:(f + 1) * 128],
                                             gact[f][:, s0],
                                             start=(f == 0), stop=(f == FFT - 1))
                        ot = opool.tile([128, 512], F32, tag="fout")
                        nc.vector.tensor_scalar(ot[:], xt[cc][:, s0], resw[:],
                                                None, op0=MULT)
                        nc.vector.tensor_tensor(ot[:], ot[:], ps[:], op=ADD)
                        nc.sync.dma_start(
                            d_out.ap()[cc * 128:(cc + 1) * 128, s0], ot[:])

            tc.no_sync_barrier()
            res_sc(b_ca[0], 0, "ca_res")
            norm_sc(0, "mlp_norm")
            mlp_up(0)
            mlp_down_out(0)
            tc.no_sync_barrier()
            res_sc(b_ca[1], 1, "ca_res")
            norm_sc(1, "mlp_norm")
            mlp_up(1)
            mlp_down_out(1)
            mpool_ctx.__exit__(None, None, None)

    nc.compile()
    return nc


# ---------------------------------------------------------------- entry
def _mask_sig(cls, pat):
    return (tuple(tuple(row) for row in cls), pat.tobytes())


def kernel(**inputs) -> np.ndarray:
    shared, per_rank, sa_cls, ca_cls = _prep_host(inputs)
    nb_sa, nb_ca = shared['mask_sa'].shape[0], shared['mask_ca'].shape[0]

    key = (_mask_sig(sa_cls, shared['mask_sa']),
           _mask_sig(ca_cls, shared['mask_ca']))
    if key not in _CACHE:
        _CACHE[key] = _build(sa_cls, ca_cls, nb_sa, nb_ca)
    nc = _CACHE[key]

    x = np.asarray(inputs['x'], dtype=np.float32)
    xb = [np.ascontiguousarray(x[g]).astype(BF_NP) for g in range(B)]
    in_maps = []
    for core in range(NCORES):
        g, r = core // TPG, core % TPG
        m = dict(shared)
        m['x'] = xb[g]
        m.update(per_rank[r])
        in_maps.append(m)

    res = run_bass_kernel_spmd(nc, in_maps, core_ids=list(range(NCORES)))
    outs = []
    for g in range(B):
        acc = np.asarray(res.results[g * TPG]['out'], dtype=np.float32).copy()
        for r in range(1, TPG):
            acc += np.asarray(res.results[g * TPG + r]['out'], dtype=np.float32)
        outs.append(acc)
    return np.stack(outs, axis=0)
